# revision 28
# baseline (speedup 1.0000x reference)
"""Trainium2 Bass kernel for nn_Attention_54717883351680.

Math: with HEADS=1 the softmax in the reference is over a size-1 axis, so
attn == 1 and the whole module collapses to

    out[0, i, j, :] = v[i] * W_out[:, 0] + b_out        (independent of j)
    v[i] = x[0, i, :] @ W_qkv[2, :]

The problem is pure memory bandwidth: write 256 MB of broadcast rows.
Sharding: queries i are split across 8 cores (256 rows each -> 32 MB/core).
Each core computes v for its queries on-chip, replicates the row vectors
into two small SBUF tiles (one per 128-query group), and streams its
contiguous 32 MB output shard to DRAM with one large DMA per group per
HWDGE ring, whose source AP uses stride-0 (broadcast) repeats of the tile.
Two concurrent rings sustain ~400 GB/s/core vs ~346 for one.

Head-latency tricks:
- consts reach all 128 partitions via a 1-descriptor DMA to partition 0
  plus a K=1 ones-matmul broadcast into PSUM (faster than a 128-descriptor
  partition-broadcast DMA).
- no nc.Block: engines reach the NEFF epilogue independently.
- narrow kernel semaphore range (smaller preamble sem_clear).
"""

import numpy as np

import concourse.bass as bass
import concourse.mybir as mybir
from concourse.bass_utils import run_bass_kernel_spmd

# This kernel uses <16 semaphores; the default range(150, 256) just adds
# preamble sem_clear work.
bass.get_kernel_semaphore_range = lambda: range(150, 170)

# Problem shape (hardcoded; kernel.py must be self-contained).
B, L, DIM = 1, 2048, 16
N_CORES = 8
QS = L // N_CORES          # queries per core = 256
P = 128                    # SBUF partitions
G = QS // P                # query groups per core = 2
J0 = 64                    # j-replication materialized in SBUF
R = L // J0                # stride-0 repeats per output DMA
E = J0 * DIM               # free elems per (partition, rep)
F32 = mybir.dt.float32

# SDMA engine 15 (serving partitions {92-95, 124-127} per the port swizzle)
# is intermittently ~15-20us slower than the other 15 engines and then gates
# the whole stream. Relief: the main full-width DMA only covers j < JCUT;
# follow-up subset DMAs (partitions 0-92 and 96-124 -> engines 0-14 only)
# cover the j-tail for healthy partitions, and the j-tail of engine-15's 16
# rows is written from a relief tile spread one-row-per-partition across the
# even engines (2 rows each). Per-engine ring FIFO keeps every engine busy:
# engine 15 simply runs out of descriptors early.
JCUT = 1536                # relieved j boundary
RCUT = JCUT // J0          # full-width reps = 24
RT = R - RCUT              # tail reps = 8
ERL = (L - JCUT) * DIM     # relief elems per partition = 8192
PT = [0, 1, 4, 5, 8, 9, 12, 13, 16, 17, 20, 21, 24, 25, 28, 29]
RELIEF_BASES = (92, 124)

_cache = {}


def _build_nc():
    nc = bass.Bass()
    xs = nc.declare_dram_parameter("xs", [QS, DIM], F32, isOutput=False)
    xr = nc.declare_dram_parameter("xr", [32, DIM], F32, isOutput=False)
    cs = nc.declare_dram_parameter("consts", [3, DIM], F32, isOutput=False)
    out = nc.declare_dram_parameter("out", [QS, L * DIM], F32, isOutput=True)

    import contextlib
    ctx = contextlib.ExitStack()
    with ctx:
        def sb(name, shape):
            return ctx.enter_context(nc.sbuf_tensor(name, shape, F32))

        def sem(name):
            return ctx.enter_context(nc.semaphore(name))

        xsb = sb("xsb", [P, G * DIM])      # [p, g*16+d] = x[g*128+p, d]
        csb0 = sb("csb0", [1, 3 * DIM])    # consts on partition 0
        ones = sb("ones", [1, P])          # matmul-broadcast lhsT
        cps = ctx.enter_context(nc.psum_tensor("cps", [P, 3 * DIM], F32))
        boutsb = sb("boutsb", [P, DIM])    # b_out in SBUF
        vsb = sb("vsb", [P, G])            # v[p, g]
        rowsb = sb("rowsb", [P, G * DIM])  # row[p, g*16+d]
        rep0 = sb("rep0", [P, E])
        rep1 = sb("rep1", [P, E])
        xrsb = sb("xrsb", [32, DIM])       # relief x rows at PT[i]
        vrel = sb("vrel", [32, 1])
        rowr = sb("rowr", [32, DIM])
        rtile = sb("rtile", [32, ERL])
        xsem, x2sem, csem, msem = sem("xsem"), sem("x2sem"), sem("csem"), sem("msem")
        psem, bsem, vsem, osem = sem("psem"), sem("bsem"), sem("vsem"), sem("osem")
        reps = (rep0, rep1)
        wv_ps = cps[:][:, 0:DIM]
        wout_ps = cps[:][:, DIM:2 * DIM]
        bout_ps = cps[:][:, 2 * DIM:3 * DIM]

        # --- input loads (two HWDGE rings in parallel) ---
        nc.sync.dma_start(
            xsb[:].rearrange("p (g d) -> p g d", d=DIM),
            xs[:].rearrange("(g p) d -> p g d", p=P),
        ).then_inc(xsem, 16)
        nc.scalar.dma_start(
            csb0[:], cs[:].rearrange("k d -> (k d)")[None, :]
        ).then_inc(csem, 16)
        nc.scalar.dma_start(xrsb[:], xr[:]).then_inc(x2sem, 16)

        # --- broadcast consts to all partitions: cps = ones.T @ csb0 ---
        nc.gpsimd.memset(ones[:], 1.0).then_inc(msem, 1)
        nc.tensor.wait_ge(csem, 16)
        nc.tensor.wait_ge(msem, 1)
        nc.tensor.matmul(
            cps[:], ones[:], csb0[:], start=True, stop=True
        ).then_inc(psem, 1)

        # --- DVE chain (explicit RAW semaphore edges; no SBUF scoreboard) ---
        nc.vector.wait_ge(psem, 1)
        nc.vector.tensor_copy(boutsb[:], bout_ps).then_inc(bsem, 1)
        nc.vector.wait_ge(xsem, 16)
        # v[p, g] = sum_d x[p, g, d] * wv[d]   (accum_out does the reduce)
        for g in range(G):
            nc.vector.scalar_tensor_tensor(
                rowsb[:, g * DIM:(g + 1) * DIM],   # scratch, overwritten below
                xsb[:, g * DIM:(g + 1) * DIM],
                1.0,
                wv_ps,
                op0=mybir.AluOpType.mult,
                op1=mybir.AluOpType.mult,
                accum_out=vsb[:][:, g:g + 1],
            ).then_inc(vsem, 1)
        # row_g[d] = wout[d] * v[p, g] + bout[d], then replicate J0 times;
        # group 0 first so its output DMA can start as early as possible.
        for g in range(G):
            nc.vector.wait_ge(vsem, 1 + g)          # v_g accumulated
            nc.vector.scalar_tensor_tensor(
                rowsb[:, g * DIM:(g + 1) * DIM],
                wout_ps,
                vsb[:][:, g:g + 1],
                boutsb[:],
                op0=mybir.AluOpType.mult,
                op1=mybir.AluOpType.add,
            ).then_inc(vsem, 1)
            nc.vector.wait_ge(vsem, 3 + 2 * g)      # row_g written
            nc.vector.tensor_copy(
                reps[g][:].rearrange("p (r d) -> p r d", d=DIM),
                rowsb[:][:, g * DIM:(g + 1) * DIM][:, None, :]
                .broadcast_to((P, J0, DIM)),
            ).then_inc(vsem, 1)

        # relief rows: v, row, replicate (j-tail content for engine-15 rows)
        nc.vector.wait_ge(x2sem, 16)
        nc.vector.scalar_tensor_tensor(
            rowr[:], xrsb[:], 1.0, wv_ps[0:32, :],
            op0=mybir.AluOpType.mult, op1=mybir.AluOpType.mult,
            accum_out=vrel[:],
        ).then_inc(vsem, 1)
        nc.vector.wait_ge(vsem, 7)
        nc.vector.scalar_tensor_tensor(
            rowr[:], wout_ps[0:32, :], vrel[:], boutsb[:][0:32, :],
            op0=mybir.AluOpType.mult, op1=mybir.AluOpType.add,
        ).then_inc(vsem, 1)
        nc.vector.wait_ge(vsem, 8)
        nc.vector.tensor_copy(
            rtile[:].rearrange("p (r d) -> p r d", d=DIM),
            rowr[:][:, None, :].broadcast_to((32, ERL // DIM, DIM)),
        ).then_inc(vsem, 1)

        # --- output stream: asymmetric 24/8 split across the two HWDGE
        # rings: full dual-queue rate early, then single-queue demand (under
        # the HBM fair share) so paired cores stop fighting late in the run.
        engines = (nc.sync, nc.scalar)
        n_dma = 0
        for g in range(G):
            eng = engines[g]
            eng.wait_ge(vsem, 4 + 2 * g)  # rep_g ready
            # full-width head: every engine busy, j in [0, JCUT)
            eng.dma_start(
                out[:][g * P:(g + 1) * P, 0:RCUT * E]
                .rearrange("p (r e) -> p r e", e=E),
                reps[g][:][:, None, :].broadcast_to((P, RCUT, E)),
            ).then_inc(osem, 16)
            # j-tail for healthy partitions only (engines 0-14)
            for (p0, p1) in ((0, RELIEF_BASES[0]), (RELIEF_BASES[0] + 4,
                                                    RELIEF_BASES[1])):
                eng.dma_start(
                    out[:][g * P + p0:g * P + p1, RCUT * E:]
                    .rearrange("p (r e) -> p r e", e=E),
                    reps[g][:][p0:p1, None, :].broadcast_to((p1 - p0, RT, E)),
                ).then_inc(osem, 16)
            n_dma += 3
        # j-tail of engine-15's rows from the relief tile (1 row/partition,
        # two rows per even engine)
        for g in range(G):
            eng = engines[g]
            eng.wait_ge(vsem, 9)  # rtile ready
            for m, base in enumerate(RELIEF_BASES):
                for i in range(4):
                    u = PT[g * 8 + m * 4 + i]
                    row = g * P + base + i
                    eng.dma_start(
                        out[:][row:row + 1, RCUT * E:],
                        rtile[:][u:u + 1, :],
                    ).then_inc(osem, 16)
                    n_dma += 1
        nc.sync.wait_ge(osem, 16 * n_dma)

    return nc


def _get_nc():
    if "nc" not in _cache:
        _cache["nc"] = _build_nc()
    return _cache["nc"]


def run(x, W_qkv, W_out, b_out, trace=False):
    nc = _get_nc()
    consts = np.ascontiguousarray(
        np.stack([W_qkv[2, :], W_out[:, 0], b_out]).astype(np.float32)
    )
    in_maps = []
    for c in range(N_CORES):
        xs_c = np.ascontiguousarray(x[0, c * QS:(c + 1) * QS, :], dtype=np.float32)
        xr_c = np.zeros((32, DIM), dtype=np.float32)
        for i in range(16):
            g, m, ii = i // 8, (i % 8) // 4, i % 4
            xr_c[PT[i]] = xs_c[g * P + RELIEF_BASES[m] + ii]
        in_maps.append({"xs": xs_c, "xr": xr_c, "consts": consts})
    res = run_bass_kernel_spmd(nc, in_maps, list(range(N_CORES)), trace=trace)
    shards = [res.results[c]["out"].reshape(QS, L, DIM) for c in range(N_CORES)]
    full = np.concatenate(shards, axis=0)[None]  # (1, 2048, 2048, 16)
    return full, res.exec_time_ns


def kernel(x, W_qkv, W_out, b_out):
    out, _ = run(x, W_qkv, W_out, b_out, trace=False)
    return out


# revision 29
# speedup vs baseline: 1.5032x; 1.5032x over previous
"""Trainium2 Bass kernel for nn_Attention_54717883351680.

Math: with HEADS=1 the softmax in the reference is over a size-1 axis, so
attn == 1 and the whole module collapses to

    out[0, i, j, :] = v[i] * W_out[:, 0] + b_out        (independent of j)
    v[i] = x[0, i, :] @ W_qkv[2, :]

The problem is pure memory bandwidth: write 256 MB of broadcast rows.
Sharding: queries i are split across 8 cores (256 rows each -> 32 MB/core).
Each core computes v for its queries on-chip, replicates the row vectors
into two small SBUF tiles (one per 128-query group), and streams its
contiguous 32 MB output shard to DRAM with large DMAs whose source APs use
stride-0 (broadcast) repeats of the tiles. Driving BOTH HWDGE rings
concurrently sustains ~400 GB/s/core vs ~346 for a single ring; the split
is 24/8 MB so the late phase falls back to single-ring demand (below the
HBM pair fair-share, calming cross-core contention).

Head-latency tricks:
- consts reach all 128 partitions via a 1-descriptor DMA to partition 0
  plus a K=1 ones-matmul broadcast into PSUM (faster than a 128-descriptor
  partition-broadcast DMA).
- no nc.Block: engines reach the NEFF epilogue independently (the exit
  barrier would otherwise serialize ~5us of per-engine semaphore restores
  after the stream).
- narrow kernel semaphore range (smaller preamble sem_clear).
"""

import numpy as np

import concourse.bass as bass
import concourse.mybir as mybir
from concourse.bass_utils import run_bass_kernel_spmd

# This kernel uses <16 semaphores; the default range(150, 256) just adds
# preamble sem_clear work.
bass.get_kernel_semaphore_range = lambda: range(150, 170)

# Problem shape (hardcoded; kernel.py must be self-contained).
B, L, DIM = 1, 2048, 16
N_CORES = 8
QS = L // N_CORES          # queries per core = 256
P = 128                    # SBUF partitions
G = QS // P                # query groups per core = 2
J0 = 64                    # j-replication materialized in SBUF
R = L // J0                # stride-0 repeats per output DMA
E = J0 * DIM               # free elems per (partition, rep)
F32 = mybir.dt.float32

_cache = {}


def _build_nc():
    nc = bass.Bass()
    xs = nc.declare_dram_parameter("xs", [QS, DIM], F32, isOutput=False)
    cs = nc.declare_dram_parameter("consts", [3, DIM], F32, isOutput=False)
    out = nc.declare_dram_parameter("out", [QS, L * DIM], F32, isOutput=True)

    with (
        nc.sbuf_tensor([P, G * DIM], F32) as xsb,     # [p, g*16+d] = x[g*128+p, d]
        nc.sbuf_tensor([1, 3 * DIM], F32) as csb0,    # consts on partition 0
        nc.sbuf_tensor([1, P], F32) as ones,          # matmul-broadcast lhsT
        nc.psum_tensor([P, 3 * DIM], F32) as cps,     # consts on all partitions
        nc.sbuf_tensor([P, DIM], F32) as boutsb,      # b_out in SBUF
        nc.sbuf_tensor([P, G], F32) as vsb,           # v[p, g]
        nc.sbuf_tensor([P, G * DIM], F32) as rowsb,   # row[p, g*16+d]
        nc.sbuf_tensor([P, E], F32) as rep0,
        nc.sbuf_tensor([P, E], F32) as rep1,
        nc.semaphore() as xsem,
        nc.semaphore() as csem,
        nc.semaphore() as msem,
        nc.semaphore() as psem,
        nc.semaphore() as bsem,
        nc.semaphore() as vsem,
        nc.semaphore() as osem,
    ):
        reps = (rep0, rep1)
        wv_ps = cps[:][:, 0:DIM]
        wout_ps = cps[:][:, DIM:2 * DIM]
        bout_ps = cps[:][:, 2 * DIM:3 * DIM]

        # --- input loads (two HWDGE rings in parallel) ---
        nc.sync.dma_start(
            xsb[:].rearrange("p (g d) -> p g d", d=DIM),
            xs[:].rearrange("(g p) d -> p g d", p=P),
        ).then_inc(xsem, 16)
        nc.scalar.dma_start(
            csb0[:], cs[:].rearrange("k d -> (k d)")[None, :]
        ).then_inc(csem, 16)

        # --- broadcast consts to all partitions: cps = ones.T @ csb0 ---
        nc.gpsimd.memset(ones[:], 1.0).then_inc(msem, 1)
        nc.tensor.wait_ge(csem, 16)
        nc.tensor.wait_ge(msem, 1)
        nc.tensor.matmul(
            cps[:], ones[:], csb0[:], start=True, stop=True
        ).then_inc(psem, 1)

        # --- DVE chain (explicit RAW semaphore edges; no SBUF scoreboard) ---
        nc.vector.wait_ge(psem, 1)
        nc.vector.tensor_copy(boutsb[:], bout_ps).then_inc(bsem, 1)
        nc.vector.wait_ge(xsem, 16)
        # v[p, g] = sum_d x[p, g, d] * wv[d]   (accum_out does the reduce)
        for g in range(G):
            nc.vector.scalar_tensor_tensor(
                rowsb[:, g * DIM:(g + 1) * DIM],   # scratch, overwritten below
                xsb[:, g * DIM:(g + 1) * DIM],
                1.0,
                wv_ps,
                op0=mybir.AluOpType.mult,
                op1=mybir.AluOpType.mult,
                accum_out=vsb[:][:, g:g + 1],
            ).then_inc(vsem, 1)
        # row_g[d] = wout[d] * v[p, g] + bout[d], then replicate J0 times;
        # group 0 first so its output DMA can start as early as possible.
        for g in range(G):
            nc.vector.wait_ge(vsem, 1 + g)          # v_g accumulated
            nc.vector.scalar_tensor_tensor(
                rowsb[:, g * DIM:(g + 1) * DIM],
                wout_ps,
                vsb[:][:, g:g + 1],
                boutsb[:],
                op0=mybir.AluOpType.mult,
                op1=mybir.AluOpType.add,
            ).then_inc(vsem, 1)
            nc.vector.wait_ge(vsem, 3 + 2 * g)      # row_g written
            nc.vector.tensor_copy(
                reps[g][:].rearrange("p (r d) -> p r d", d=DIM),
                rowsb[:][:, g * DIM:(g + 1) * DIM][:, None, :]
                .broadcast_to((P, J0, DIM)),
            ).then_inc(vsem, 1)

        # --- output stream: asymmetric 24/8 split across the two HWDGE
        # rings: full dual-queue rate early, then single-queue demand (under
        # the HBM pair fair-share, calming cross-core contention).
        HS = R // 4  # scalar ring carries g1 j in [0, L/4) = 8 MB
        nc.sync.wait_ge(vsem, 4)  # rep0 ready
        nc.sync.dma_start(
            out[:][0:P, :].rearrange("p (r e) -> p r e", e=E),
            rep0[:][:, None, :].broadcast_to((P, R, E)),
        ).then_inc(osem, 16)
        nc.scalar.wait_ge(vsem, 6)  # rep1 ready
        nc.scalar.dma_start(
            out[:][P:2 * P, 0:HS * E].rearrange("p (r e) -> p r e", e=E),
            rep1[:][:, None, :].broadcast_to((P, HS, E)),
        ).then_inc(osem, 16)
        nc.sync.wait_ge(vsem, 6)
        nc.sync.dma_start(
            out[:][P:2 * P, HS * E:].rearrange("p (r e) -> p r e", e=E),
            rep1[:][:, None, :].broadcast_to((P, R - HS, E)),
        ).then_inc(osem, 16)
        nc.sync.wait_ge(osem, 48)

    return nc


def _get_nc():
    if "nc" not in _cache:
        _cache["nc"] = _build_nc()
    return _cache["nc"]


def run(x, W_qkv, W_out, b_out, trace=False):
    nc = _get_nc()
    consts = np.ascontiguousarray(
        np.stack([W_qkv[2, :], W_out[:, 0], b_out]).astype(np.float32)
    )
    in_maps = [
        {
            "xs": np.ascontiguousarray(x[0, c * QS:(c + 1) * QS, :], dtype=np.float32),
            "consts": consts,
        }
        for c in range(N_CORES)
    ]
    res = run_bass_kernel_spmd(nc, in_maps, list(range(N_CORES)), trace=trace)
    shards = [res.results[c]["out"].reshape(QS, L, DIM) for c in range(N_CORES)]
    full = np.concatenate(shards, axis=0)[None]  # (1, 2048, 2048, 16)
    return full, res.exec_time_ns


def kernel(x, W_qkv, W_out, b_out):
    out, _ = run(x, W_qkv, W_out, b_out, trace=False)
    return out


# revision 30
# speedup vs baseline: 1.5372x; 1.0226x over previous
"""Trainium2 Bass kernel for nn_Attention_54717883351680.

Math: with HEADS=1 the softmax in the reference is over a size-1 axis, so
attn == 1 and the whole module collapses to

    out[0, i, j, :] = v[i] * W_out[:, 0] + b_out        (independent of j)
    v[i] = x[0, i, :] @ W_qkv[2, :]

The problem is pure memory bandwidth: write 256 MB of broadcast rows.
Sharding: queries i are split across 8 cores (256 rows each -> 32 MB/core).
Each core computes v for its queries on-chip, replicates the row vectors
into two small SBUF tiles (one per 128-query group), and streams its
contiguous 32 MB output shard to DRAM with large DMAs whose source APs use
stride-0 (broadcast) repeats of the tiles. Driving BOTH HWDGE rings
concurrently sustains ~400 GB/s/core vs ~346 for a single ring; the split
is 24/8 MB so the late phase falls back to single-ring demand (below the
HBM pair fair-share, calming cross-core contention).

Head-latency tricks:
- consts reach all 128 partitions via a 1-descriptor DMA to partition 0
  plus a K=1 ones-matmul broadcast into PSUM (faster than a 128-descriptor
  partition-broadcast DMA).
- no nc.Block: engines reach the NEFF epilogue independently (the exit
  barrier would otherwise serialize ~5us of per-engine semaphore restores
  after the stream).
- narrow kernel semaphore range (smaller preamble sem_clear).
"""

import numpy as np

import concourse.bass as bass
import concourse.mybir as mybir
from concourse.bass_utils import run_bass_kernel_spmd

# This kernel uses <16 semaphores; the default range(150, 256) just adds
# preamble sem_clear work.
bass.get_kernel_semaphore_range = lambda: range(150, 170)

# Problem shape (hardcoded; kernel.py must be self-contained).
B, L, DIM = 1, 2048, 16
N_CORES = 8
QS = L // N_CORES          # queries per core = 256
P = 128                    # SBUF partitions
G = QS // P                # query groups per core = 2
J0 = 128                   # j-replication materialized in SBUF
R = L // J0                # stride-0 repeats per output DMA
E = J0 * DIM               # free elems per (partition, rep)
F32 = mybir.dt.float32

_cache = {}


def _build_nc():
    nc = bass.Bass()
    xs = nc.declare_dram_parameter("xs", [QS, DIM], F32, isOutput=False)
    cs = nc.declare_dram_parameter("consts", [3, DIM], F32, isOutput=False)
    out = nc.declare_dram_parameter("out", [QS, L * DIM], F32, isOutput=True)

    with (
        nc.sbuf_tensor([P, G * DIM], F32) as xsb,     # [p, g*16+d] = x[g*128+p, d]
        nc.sbuf_tensor([1, 3 * DIM], F32) as csb0,    # consts on partition 0
        nc.sbuf_tensor([1, P], F32) as ones,          # matmul-broadcast lhsT
        nc.psum_tensor([P, 3 * DIM], F32) as cps,     # consts on all partitions
        nc.sbuf_tensor([P, DIM], F32) as boutsb,      # b_out in SBUF
        nc.sbuf_tensor([P, G], F32) as vsb,           # v[p, g]
        nc.sbuf_tensor([P, G * DIM], F32) as rowsb,   # row[p, g*16+d]
        nc.sbuf_tensor([P, E], F32) as rep0,
        nc.sbuf_tensor([P, E], F32) as rep1,
        nc.semaphore() as xsem,
        nc.semaphore() as csem,
        nc.semaphore() as msem,
        nc.semaphore() as psem,
        nc.semaphore() as bsem,
        nc.semaphore() as vsem,
        nc.semaphore() as osem,
    ):
        reps = (rep0, rep1)
        wv_ps = cps[:][:, 0:DIM]
        wout_ps = cps[:][:, DIM:2 * DIM]
        bout_ps = cps[:][:, 2 * DIM:3 * DIM]

        # --- input loads (two HWDGE rings in parallel) ---
        nc.sync.dma_start(
            xsb[:].rearrange("p (g d) -> p g d", d=DIM),
            xs[:].rearrange("(g p) d -> p g d", p=P),
        ).then_inc(xsem, 16)
        nc.scalar.dma_start(
            csb0[:], cs[:].rearrange("k d -> (k d)")[None, :]
        ).then_inc(csem, 16)

        # --- broadcast consts to all partitions: cps = ones.T @ csb0 ---
        nc.gpsimd.memset(ones[:], 1.0).then_inc(msem, 1)
        nc.tensor.wait_ge(csem, 16)
        nc.tensor.wait_ge(msem, 1)
        nc.tensor.matmul(
            cps[:], ones[:], csb0[:], start=True, stop=True
        ).then_inc(psem, 1)

        # --- DVE chain (explicit RAW semaphore edges; no SBUF scoreboard) ---
        nc.vector.wait_ge(psem, 1)
        nc.vector.tensor_copy(boutsb[:], bout_ps).then_inc(bsem, 1)
        nc.vector.wait_ge(xsem, 16)
        # v[p, g] = sum_d x[p, g, d] * wv[d]   (accum_out does the reduce)
        for g in range(G):
            nc.vector.scalar_tensor_tensor(
                rowsb[:, g * DIM:(g + 1) * DIM],   # scratch, overwritten below
                xsb[:, g * DIM:(g + 1) * DIM],
                1.0,
                wv_ps,
                op0=mybir.AluOpType.mult,
                op1=mybir.AluOpType.mult,
                accum_out=vsb[:][:, g:g + 1],
            ).then_inc(vsem, 1)
        # row_g[d] = wout[d] * v[p, g] + bout[d], then replicate J0 times;
        # group 0 first so its output DMA can start as early as possible.
        for g in range(G):
            nc.vector.wait_ge(vsem, 1 + g)          # v_g accumulated
            nc.vector.scalar_tensor_tensor(
                rowsb[:, g * DIM:(g + 1) * DIM],
                wout_ps,
                vsb[:][:, g:g + 1],
                boutsb[:],
                op0=mybir.AluOpType.mult,
                op1=mybir.AluOpType.add,
            ).then_inc(vsem, 1)
            nc.vector.wait_ge(vsem, 3 + 2 * g)      # row_g written
            nc.vector.tensor_copy(
                reps[g][:].rearrange("p (r d) -> p r d", d=DIM),
                rowsb[:][:, g * DIM:(g + 1) * DIM][:, None, :]
                .broadcast_to((P, J0, DIM)),
            ).then_inc(vsem, 1)

        # --- output stream: asymmetric 24/8 split across the two HWDGE
        # rings: full dual-queue rate early, then single-queue demand (under
        # the HBM pair fair-share, calming cross-core contention).
        HS = R // 4  # scalar ring carries g1 j in [0, L/4) = 8 MB
        nc.sync.wait_ge(vsem, 4)  # rep0 ready
        nc.sync.dma_start(
            out[:][0:P, :].rearrange("p (r e) -> p r e", e=E),
            rep0[:][:, None, :].broadcast_to((P, R, E)),
        ).then_inc(osem, 16)
        nc.scalar.wait_ge(vsem, 6)  # rep1 ready
        nc.scalar.dma_start(
            out[:][P:2 * P, 0:HS * E].rearrange("p (r e) -> p r e", e=E),
            rep1[:][:, None, :].broadcast_to((P, HS, E)),
        ).then_inc(osem, 16)
        nc.sync.wait_ge(vsem, 6)
        nc.sync.dma_start(
            out[:][P:2 * P, HS * E:].rearrange("p (r e) -> p r e", e=E),
            rep1[:][:, None, :].broadcast_to((P, R - HS, E)),
        ).then_inc(osem, 16)
        nc.sync.wait_ge(osem, 48)

    return nc


def _get_nc():
    if "nc" not in _cache:
        _cache["nc"] = _build_nc()
    return _cache["nc"]


def run(x, W_qkv, W_out, b_out, trace=False):
    nc = _get_nc()
    consts = np.ascontiguousarray(
        np.stack([W_qkv[2, :], W_out[:, 0], b_out]).astype(np.float32)
    )
    in_maps = [
        {
            "xs": np.ascontiguousarray(x[0, c * QS:(c + 1) * QS, :], dtype=np.float32),
            "consts": consts,
        }
        for c in range(N_CORES)
    ]
    res = run_bass_kernel_spmd(nc, in_maps, list(range(N_CORES)), trace=trace)
    shards = [res.results[c]["out"].reshape(QS, L, DIM) for c in range(N_CORES)]
    full = np.concatenate(shards, axis=0)[None]  # (1, 2048, 2048, 16)
    return full, res.exec_time_ns


def kernel(x, W_qkv, W_out, b_out):
    out, _ = run(x, W_qkv, W_out, b_out, trace=False)
    return out
